# revision 1
# baseline (speedup 1.0000x reference)
"""LoRA linear layer (out = x @ (W + B@A).T + bias) on 8 trn2 NeuronCores.

Strategy: data-parallel over tokens (B*S = 8192 -> 1024 tokens/core).
Each core computes its token-shard against the full weight:
  - x shard is DMA'd in, transposed on the PE (128x128 tiles) into a
    resident SBUF xT [d_in, T] laid out as [128, KT, T].
  - U = (x @ A.T).T = [r, T] computed once with A.T as stationary operand.
  - For each 128-row block m of W: DMA the rows, PE-transpose into
    WT [128, KT, 128]; accumulate psum[o=128, t=512] over the 32 k-tiles
    (fp32r matmuls, weight loaded once per (m,k) and reused for both
    t-chunks), then one rank-16 matmul adds the LoRA term from B.T and U.
  - psum is evicted through the Scalar engine with the bias added
    (bias laid out per-partition), PE-transposed back to [t, o] tiles and
    DMA'd out contiguously.
"""

import sys
import types

sys.path.insert(0, "/opt/trn_rl_repo")

import numpy as np

import concourse.bass as bass  # noqa: F401
import concourse.bacc as bacc
import concourse.tile as tile
from concourse import mybir, bass_utils
from concourse.masks import make_identity
from contextlib import ExitStack

P = 128
N_CORES = 8

# Full problem shapes (hardcoded per contract).
B_FULL, S_FULL, D_IN, D_OUT, R = 4, 2048, 4096, 4096, 16
T_CORE = (B_FULL * S_FULL) // N_CORES  # 1024 tokens per core


def build_nc(T=T_CORE, DIN=D_IN, DOUT=D_OUT, r=R, tr_fpr=False, w_bf16=False, fp16=False):
    """Build the per-core bass program. All cores run the same program on
    different token shards."""
    FP = mybir.dt.float32
    FPR = mybir.dt.float16 if fp16 else mybir.dt.float32r
    KT = DIN // P
    MT = DOUT // P
    NCH = min(512, T)  # moving-operand chunk (>=256 keeps fp32r at full rate)
    NT = T // NCH
    TRD = FPR if fp16 else (mybir.dt.float32r if tr_fpr else mybir.dt.float32)
    TG = 4  # transposes grouped per PSUM bank before one batched eviction
    NWCH = 4  # W row-block DMA'd in this many chunks
    HKT = KT // NWCH

    nc = bacc.Bacc("TRN2", target_bir_lowering=False, debug=False)
    x_d = nc.dram_tensor("x", [T, DIN], FP, kind="ExternalInput").ap()
    w_d = nc.dram_tensor("w", [DOUT, DIN], FP, kind="ExternalInput").ap()
    br_d = nc.dram_tensor("bias_r", [P, MT], FP, kind="ExternalInput").ap()
    at_d = nc.dram_tensor("at", [DIN, r], FP, kind="ExternalInput").ap()
    bt_d = nc.dram_tensor("bt", [r, DOUT], FP, kind="ExternalInput").ap()
    id_d = nc.dram_tensor("ident", [P, P], FP, kind="ExternalInput").ap()
    out_d = nc.dram_tensor("out", [T, DOUT], FP, kind="ExternalOutput").ap()

    with tile.TileContext(nc) as tc, ExitStack() as ctx:
        const = ctx.enter_context(tc.tile_pool(name="const", bufs=1))
        ident = const.tile([P, P], FP)
        nc.sync.dma_start(ident[:], id_d[:])
        if tr_fpr or fp16:
            ident_r = const.tile([P, P], FPR)
            nc.vector.tensor_copy(ident_r[:], ident[:])
        else:
            ident_r = ident
        bias_sb = const.tile([P, MT], FP)
        nc.sync.dma_start(bias_sb[:], br_d[:])
        at_sb = const.tile([P, KT, r], FPR)
        bt_sb = const.tile([r, DOUT], FPR)
        xt_all = const.tile([P, KT, T], FPR)  # resident x^T, 16 MB
        u_sb = const.tile([r, T], FPR)

        tp_psum = ctx.enter_context(tc.tile_pool(name="tpps", bufs=4, space="PSUM"))

        # ---- stage 1: transpose x shard into xt_all (TG tiles per bank) ----
        with tc.tile_pool(name="xrawp", bufs=4) as xraw_pool:
            at_raw = xraw_pool.tile([P, KT, r], FP, tag="xraw")
            nc.sync.dma_start(at_raw[:], at_d.rearrange("(k p) r -> p k r", p=P))
            nc.vector.tensor_copy(at_sb[:], at_raw[:])
            bt_raw = xraw_pool.tile([r, DOUT], FP, tag="xraw")
            nc.sync.dma_start(bt_raw[:], bt_d[:])
            nc.vector.tensor_copy(bt_sb[:], bt_raw[:])
            TBN = NCH // P  # token-blocks per U chunk
            for tb in range(T // P):
                tsl = slice(tb * P, (tb + 1) * P)
                xraw = xraw_pool.tile([P, DIN], FPR if fp16 else FP, tag="xraw")
                QD = DIN // 4
                for q in range(4):
                    (nc.gpsimd if fp16 else nc.sync).dma_start(
                        xraw[:, q * QD : (q + 1) * QD],
                        x_d[tsl, q * QD : (q + 1) * QD],
                    )
                if tr_fpr:
                    # round fp32 -> fp32r in place (same bytes budget)
                    for q in range(4):
                        nc.vector.tensor_copy(
                            xraw[:, q * QD : (q + 1) * QD].bitcast(FPR),
                            xraw[:, q * QD : (q + 1) * QD],
                        )
                    xsrc = xraw.bitcast(FPR)
                else:
                    xsrc = xraw
                for g in range(KT // TG):
                    ps = tp_psum.tile([P, TG * P], TRD, tag="tp", name="psx")
                    for j in range(TG):
                        k = g * TG + j
                        nc.tensor.transpose(
                            ps[:, j * P : (j + 1) * P],
                            xsrc[:, k * P : (k + 1) * P],
                            ident_r[:],
                        )
                    psv = ps[:] if fp16 else ps[:].bitcast(FP)
                    nc.vector.tensor_copy(
                        xt_all[:, g * TG : (g + 1) * TG, tsl],
                        psv.rearrange("p (j q) -> p j q", j=TG),
                    )
                if tb % TBN == TBN - 1:
                    # U chunk for these token-blocks: U[r, nsl] = A @ x^T
                    n = tb // TBN
                    nsl = slice(n * NCH, (n + 1) * NCH)
                    ups = tp_psum.tile([r, NCH], FP, tag="tp", name="ups")
                    for k in range(KT):
                        nc.tensor.matmul(
                            ups[:],
                            at_sb[:, k, :],
                            xt_all[:, k, nsl],
                            start=(k == 0),
                            stop=(k == KT - 1),
                        )
                    nc.vector.tensor_copy(u_sb[:, nsl], ups[:])

        # ---- stage 3: main loop; next block's W transposes interleave with
        # the current block's matmuls so weight loads hide under them ----
        wraw_pool = ctx.enter_context(tc.tile_pool(name="wrawp", bufs=4 if fp16 else 2))
        wt_pool = ctx.enter_context(tc.tile_pool(name="wtp", bufs=3 if fp16 else 2))
        mm_psum = ctx.enter_context(tc.tile_pool(name="mmps", bufs=2, space="PSUM"))
        ob_pool = ctx.enter_context(tc.tile_pool(name="obp", bufs=6))
        ot_pool = ctx.enter_context(tc.tile_pool(name="otp", bufs=6))

        wstate = {}

        def w_step(m_next, k):
            """Emit DMA/transpose/evict steps for tile k of W row-block m_next."""
            if k % HKT == 0:
                h = k // HKT
                wraw = wraw_pool.tile(
                    [P, HKT * P], FPR if fp16 else FP, tag="wraw", name="wraw"
                )
                (nc.gpsimd if fp16 else nc.sync).dma_start(
                    wraw[:],
                    w_d[m_next * P : (m_next + 1) * P, h * HKT * P : (h + 1) * HKT * P],
                )
                if tr_fpr:
                    nc.vector.tensor_copy(wraw[:].bitcast(FPR), wraw[:])
                    wstate["wraw"] = wraw.bitcast(FPR)
                else:
                    wstate["wraw"] = wraw
            if k % TG == 0:
                wstate["ps"] = tp_psum.tile([P, TG * P], TRD, tag="tp", name="psw")
            kk = k % HKT
            nc.tensor.transpose(
                wstate["ps"][:, (k % TG) * P : (k % TG + 1) * P],
                wstate["wraw"][:, kk * P : (kk + 1) * P],
                ident_r[:],
            )
            if k % TG == TG - 1:
                g = k // TG
                wpsv = wstate["ps"][:] if fp16 else wstate["ps"][:].bitcast(FP)
                nc.vector.tensor_copy(
                    wstate["wt"][:, g * TG : (g + 1) * TG, :],
                    wpsv.rearrange("p (j q) -> p j q", j=TG),
                )

        # prologue: build wt for m=0
        wstate["wt"] = wt_pool.tile([P, KT, P], FPR, tag="wt", name="wt0")
        for k in range(KT):
            w_step(0, k)

        for m in range(MT):
            msl = slice(m * P, (m + 1) * P)
            wt_cur = wstate["wt"]
            if m + 1 < MT:
                wstate["wt"] = wt_pool.tile([P, KT, P], FPR, tag="wt", name="wtn")

            mps = [
                mm_psum.tile([P, NCH], FP, tag=f"mm{n}", name=f"mps{n}")
                for n in range(NT)
            ]
            for k in range(KT):
                if m + 1 < MT:
                    w_step(m + 1, k)
                for n in range(NT):
                    nc.tensor.matmul(
                        mps[n][:],
                        wt_cur[:, k, :],
                        xt_all[:, k, n * NCH : (n + 1) * NCH],
                        start=(k == 0),
                        stop=False,
                    )
            for n in range(NT):
                nsl = slice(n * NCH, (n + 1) * NCH)
                nc.tensor.matmul(
                    mps[n][:],
                    bt_sb[:r, msl],
                    u_sb[:r, nsl],
                    start=False,
                    stop=True,
                )
                ob = ob_pool.tile([P, NCH], FP, tag="ob", name="ob")
                nc.scalar.activation(
                    ob[:],
                    mps[n][:],
                    mybir.ActivationFunctionType.Identity,
                    bias=bias_sb[:, m : m + 1],
                )
                # DVE 32x32 block-transpose + block-swizzled DMA writes the
                # [o, t] tile to [t, o] DRAM with no tensor-engine work.
                ot = ot_pool.tile([P, NCH], FP, tag="otsb", name="ot")
                nc.vector.transpose(ot[:], ob[:])
                for bb in range(P // 32):
                    o0 = m * P + bb * 32
                    dst = out_d[nsl, o0 : o0 + 32].rearrange(
                        "(C u) v -> u C v", u=32
                    )
                    eng = (nc.sync, nc.gpsimd, nc.scalar, nc.gpsimd)[bb]
                    eng.dma_start(dst, ot[bb * 32 : (bb + 1) * 32, :])

    nc.compile()
    return nc


def make_in_maps(x, weight, bias, lora_A, lora_B):
    xf = np.ascontiguousarray(x.reshape(-1, x.shape[-1]), dtype=np.float32)
    T = xf.shape[0] // N_CORES
    MT = weight.shape[0] // P
    w = np.ascontiguousarray(weight, dtype=np.float32)
    bias_r = np.ascontiguousarray(
        bias.astype(np.float32).reshape(MT, P).T
    )
    at = np.ascontiguousarray(lora_A.astype(np.float32).T)
    bt = np.ascontiguousarray(lora_B.astype(np.float32).T)
    ident = np.eye(P, dtype=np.float32)
    return [
        {
            "x": np.ascontiguousarray(xf[c * T : (c + 1) * T]),
            "w": w,
            "bias_r": bias_r,
            "at": at,
            "bt": bt,
            "ident": ident,
        }
        for c in range(N_CORES)
    ]


_nc_cache = {}


def kernel(x, weight, bias, lora_A, lora_B):
    key = (x.shape, weight.shape)
    if key not in _nc_cache:
        _nc_cache[key] = build_nc(fp16=True)
    nc = _nc_cache[key]
    in_maps = make_in_maps(x, weight, bias, lora_A, lora_B)
    res = bass_utils.run_bass_kernel_spmd(
        nc, in_maps, core_ids=list(range(N_CORES))
    )
    out = np.concatenate([res.results[c]["out"] for c in range(N_CORES)], axis=0)
    return out.reshape(x.shape[:-1] + (weight.shape[0],))


if __name__ == "__main__":
    rng = np.random.default_rng(0)
    x = rng.standard_normal((B_FULL, S_FULL, D_IN), dtype=np.float32)
    w = (rng.standard_normal((D_OUT, D_IN), dtype=np.float32) * 0.02).astype(np.float32)
    b = (rng.standard_normal((D_OUT,), dtype=np.float32) * 0.02).astype(np.float32)
    la = (rng.standard_normal((R, D_IN), dtype=np.float32) * 0.02).astype(np.float32)
    lb = (rng.standard_normal((D_OUT, R), dtype=np.float32) * 0.02).astype(np.float32)
    out = kernel(x, w, b, la, lb)
    ref = x.reshape(-1, D_IN) @ (w + lb @ la).T + b
    err = np.abs(out.reshape(-1, D_OUT) - ref)
    denom = np.abs(ref).max()
    print("max abs err:", err.max(), "rel:", err.max() / denom)



# revision 9
# speedup vs baseline: 1.2306x; 1.2306x over previous
"""LoRA linear layer (out = x @ (W + B@A).T + bias) on 8 trn2 NeuronCores.

Strategy: data-parallel over tokens (B*S = 8192 -> 1024 tokens/core).
All layout work (transposes, dtype casts) happens on the host so the
device does nothing but matmuls:

  - x shard arrives pre-transposed: a fp8(e4m3) copy of the first K8
    contraction dims in DoubleRow pair layout, and a fp16 copy of the
    remaining dims. Both DMA straight into resident SBUF.
  - W arrives pre-transposed (k-major) and pre-cast: fp8 (DoubleRow
    [SW-interleave] pair layout, scaled by 64) for k < K8, fp16
    (scaled by 64) for k >= K8. Streamed per 128-row output block.
  - Per output block m and token chunk n: psum[o=128, t=512]
    accumulates G8 DoubleRow fp8 matmuls (256 k-dims each, 2x rate),
    KT16 fp16 matmuls (128 k-dims each), and one rank-16 LoRA matmul
    from B.T and U = (64*A) @ x.T.
  - Scalar engine evicts psum with the bias added and the 1/64 weight
    scale removed; plain contiguous DMA writes out.T [o, t] rows and
    the host transposes each shard back.

fp8 on only half the contraction dims keeps the max relative error
~1.8e-2 (measured on the reference data) while cutting PE time ~25%.
"""

import sys

sys.path.insert(0, "/opt/trn_rl_repo")

import numpy as np
import ml_dtypes

import concourse.bass as bass  # noqa: F401
import concourse.bacc as bacc
import concourse.tile as tile
from concourse import mybir, bass_utils
from contextlib import ExitStack

P = 128
N_CORES = 8

# Full problem shapes (hardcoded per contract).
B_FULL, S_FULL, D_IN, D_OUT, R = 4, 2048, 4096, 4096, 16
T_CORE = (B_FULL * S_FULL) // N_CORES  # 1024 tokens per core
MT = D_OUT // P  # 32 output row blocks
NCH = 512  # token chunk (one psum bank)
NT = T_CORE // NCH  # 2 chunks
K8 = 2048  # leading contraction dims done in fp8 DoubleRow
SCALE = 64.0  # fp8 weight scale (keeps 64*W in e4m3 normal range)
SWIL = True  # DoubleRowSwInterleave (contiguous ldweights) vs DoubleRow

FP8NP = ml_dtypes.float8_e4m3fn


def build_nc(T=T_CORE, k8=K8, swil=SWIL, fp16=None):
    """Per-core bass program; all cores run it on different token shards.

    fp16 kwarg is accepted for test.py compat: fp16=True/None keeps the
    default hybrid config, fp16-only can be forced with k8=0.
    """
    F32 = mybir.dt.float32
    F16 = mybir.dt.float16
    F8 = mybir.dt.float8e4
    G8 = k8 // 256
    KT16 = (D_IN - k8) // P
    DRMODE = (
        mybir.MatmulPerfMode.DoubleRowSwInterleave
        if swil
        else mybir.MatmulPerfMode.DoubleRow
    )
    IDENT = mybir.ActivationFunctionType.Identity
    HT = T // 2

    nc = bacc.Bacc("TRN2", target_bir_lowering=False, debug=False)
    if G8:
        x8_d = nc.dram_tensor("x8", [P, G8, 2, T], F8, kind="ExternalInput").ap()
        if swil:
            w8_d = nc.dram_tensor("w8", [MT, P, G8, 2 * P], F8, kind="ExternalInput").ap()
        else:
            w8_d = nc.dram_tensor("w8", [MT, P, G8, 2, P], F8, kind="ExternalInput").ap()
        at8_d = nc.dram_tensor("at8", [P, G8, 2, R], F8, kind="ExternalInput").ap()
    if KT16:
        x16_d = nc.dram_tensor("x16", [P, KT16, T], F16, kind="ExternalInput").ap()
        w16_d = nc.dram_tensor("w16", [MT, P, KT16, P], F16, kind="ExternalInput").ap()
        at16_d = nc.dram_tensor("at16", [P, KT16, R], F16, kind="ExternalInput").ap()
    bt_d = nc.dram_tensor("bt", [R, D_OUT], F16, kind="ExternalInput").ap()
    bias_d = nc.dram_tensor("bias_r", [P, MT], F32, kind="ExternalInput").ap()
    out_d = nc.dram_tensor("out", [D_OUT, T], F32, kind="ExternalOutput").ap()

    with tile.TileContext(nc) as tc, ExitStack() as ctx:
        const = ctx.enter_context(tc.tile_pool(name="const", bufs=1))
        if G8:
            xt8 = const.tile([P, G8, 2, T], F8)
            at8s = const.tile([P, G8, 2, R], F8)
        if KT16:
            xt16 = const.tile([P, KT16, T], F16)
            at16s = const.tile([P, KT16, R], F16)
        bt_sb = const.tile([R, D_OUT], F16)
        bias_sb = const.tile([P, MT], F32)
        u_sb = const.tile([R, T], F16)

        # ---- prologue DMAs: x halves spread over four queues ----
        if KT16:
            nc.sync.dma_start(xt16[:, :, 0:HT], x16_d[:, :, 0:HT])
            nc.gpsimd.dma_start(xt16[:, :, HT:T], x16_d[:, :, HT:T])
            nc.sync.dma_start(at16s[:], at16_d[:])
        if G8:
            nc.scalar.dma_start(xt8[:, :, :, 0:HT], x8_d[:, :, :, 0:HT])
            nc.scalar.dma_start(xt8[:, :, :, HT:T], x8_d[:, :, :, HT:T])
            nc.scalar.dma_start(at8s[:], at8_d[:])
        nc.sync.dma_start(bt_sb[:], bt_d[:])
        nc.sync.dma_start(bias_sb[:], bias_d[:])

        up_psum = ctx.enter_context(tc.tile_pool(name="upps", bufs=2, space="PSUM"))
        mm_psum = ctx.enter_context(tc.tile_pool(name="mmps", bufs=4, space="PSUM"))
        w8_pool = ctx.enter_context(tc.tile_pool(name="w8p", bufs=3))
        w16_pool = ctx.enter_context(tc.tile_pool(name="w16p", bufs=3))
        ob_pool = ctx.enter_context(tc.tile_pool(name="obp", bufs=4))

        # ---- U = (64*A) @ x^T, [R, T] ----
        for n in range(NT):
            nsl = slice(n * NCH, (n + 1) * NCH)
            ups = up_psum.tile([R, NCH], F32, tag="up", name="ups")
            first = True
            for g in range(G8):
                for s_ in range(2):
                    nc.tensor.matmul(
                        ups[:],
                        at8s[:, g, s_, :],
                        xt8[:, g, s_, nsl],
                        start=first,
                        stop=(not KT16) and g == G8 - 1 and s_ == 1,
                    )
                    first = False
            for j in range(KT16):
                nc.tensor.matmul(
                    ups[:],
                    at16s[:, j, :],
                    xt16[:, j, nsl],
                    start=first,
                    stop=j == KT16 - 1,
                )
                first = False
            nc.vector.tensor_copy(u_sb[:, nsl], ups[:])

        # ---- main loop over output row blocks, W prefetched one ahead ----
        def dma_w(m):
            t8 = t16 = None
            if G8:
                t8 = w8_pool.tile(
                    [P, G8, 2 * P] if swil else [P, G8, 2, P], F8, tag="w8", name="w8t"
                )
                nc.sync.dma_start(t8[:], w8_d[m])
            if KT16:
                t16 = w16_pool.tile([P, KT16, P], F16, tag="w16", name="w16t")
                nc.sync.dma_start(t16[:], w16_d[m])
            return t8, t16

        cur = dma_w(0)
        for m in range(MT):
            nxt = dma_w(m + 1) if m + 1 < MT else None
            w8t, w16t = cur
            msl = slice(m * P, (m + 1) * P)
            for n in range(NT):
                nsl = slice(n * NCH, (n + 1) * NCH)
                ps = mm_psum.tile([P, NCH], F32, tag="mm", name="mps")
                for g in range(G8):
                    w_ap = w8t[:, g, :] if swil else w8t[:, g, :, :]
                    nc.tensor.matmul(
                        ps[:],
                        w_ap,
                        xt8[:, g, :, nsl],
                        start=g == 0,
                        stop=False,
                        perf_mode=DRMODE,
                    )
                for j in range(KT16):
                    nc.tensor.matmul(
                        ps[:],
                        w16t[:, j, :],
                        xt16[:, j, nsl],
                        start=(not G8) and j == 0,
                        stop=False,
                    )
                nc.tensor.matmul(
                    ps[:], bt_sb[:, msl], u_sb[:, nsl], start=False, stop=True
                )
                ob = ob_pool.tile([P, NCH], F32, tag="ob", name="ob")
                nc.scalar.activation(
                    ob[:], ps[:], IDENT, bias=bias_sb[:, m : m + 1], scale=1.0 / SCALE
                )
                (nc.scalar if n == 0 else nc.gpsimd).dma_start(out_d[msl, nsl], ob[:])
            cur = nxt

    nc.compile()
    return nc


def _fp8(a):
    return np.clip(a, -240.0, 240.0).astype(FP8NP)


def make_in_maps(x, weight, bias, lora_A, lora_B, k8=K8, swil=SWIL):
    G8 = k8 // 256
    KT16 = (D_IN - k8) // P
    T = T_CORE
    xf = np.ascontiguousarray(x.reshape(-1, D_IN), dtype=np.float32)

    # ---- shared (per-core identical) weight-side arrays ----
    shared = {}
    w = np.asarray(weight, dtype=np.float32)
    a = np.asarray(lora_A, dtype=np.float32)
    if G8:
        wq8 = _fp8(SCALE * w[:, :k8]).reshape(MT, P, G8, 2, P)
        if swil:
            # [m, p, g, c_rev, s] pairs, contiguous for SW-interleaved ldweights
            shared["w8"] = np.ascontiguousarray(
                wq8[:, ::-1].transpose(0, 4, 2, 1, 3).reshape(MT, P, G8, 2 * P)
            )
        else:
            shared["w8"] = np.ascontiguousarray(wq8.transpose(0, 4, 2, 3, 1))
        aq8 = _fp8(SCALE * a[:, :k8]).T.reshape(G8, 2, P, R)
        shared["at8"] = np.ascontiguousarray(aq8.transpose(2, 0, 1, 3))
    if KT16:
        w16 = (SCALE * w[:, k8:]).astype(np.float16).reshape(MT, P, KT16, P)
        shared["w16"] = np.ascontiguousarray(w16.transpose(0, 3, 2, 1))
        a16 = (SCALE * a[:, k8:]).astype(np.float16).T.reshape(KT16, P, R)
        shared["at16"] = np.ascontiguousarray(a16.transpose(1, 0, 2))
    shared["bt"] = np.ascontiguousarray(
        np.asarray(lora_B, dtype=np.float32).T.astype(np.float16)
    )
    shared["bias_r"] = np.ascontiguousarray(
        np.asarray(bias, dtype=np.float32).reshape(MT, P).T
    )

    # ---- per-core token shards ----
    maps = []
    for c in range(N_CORES):
        xs = np.ascontiguousarray(xf[c * T : (c + 1) * T].T)  # [D_IN, T]
        m = dict(shared)
        if G8:
            m["x8"] = np.ascontiguousarray(
                _fp8(xs[:k8]).reshape(G8, 2, P, T).transpose(2, 0, 1, 3)
            )
        if KT16:
            m["x16"] = np.ascontiguousarray(
                xs[k8:].astype(np.float16).reshape(KT16, P, T).transpose(1, 0, 2)
            )
        maps.append(m)
    return maps


_nc_cache = {}


def kernel(x, weight, bias, lora_A, lora_B):
    key = (x.shape, weight.shape)
    if key not in _nc_cache:
        _nc_cache[key] = build_nc()
    nc = _nc_cache[key]
    in_maps = make_in_maps(x, weight, bias, lora_A, lora_B)
    res = bass_utils.run_bass_kernel_spmd(
        nc, in_maps, core_ids=list(range(N_CORES))
    )
    out = np.concatenate(
        [res.results[c]["out"].T for c in range(N_CORES)], axis=0
    )
    return out.reshape(x.shape[:-1] + (weight.shape[0],))


if __name__ == "__main__":
    rng = np.random.default_rng(0)
    x = rng.standard_normal((B_FULL, S_FULL, D_IN), dtype=np.float32)
    w = (rng.standard_normal((D_OUT, D_IN), dtype=np.float32) * 0.02).astype(np.float32)
    b = (rng.standard_normal((D_OUT,), dtype=np.float32) * 0.02).astype(np.float32)
    la = (rng.standard_normal((R, D_IN), dtype=np.float32) * 0.02).astype(np.float32)
    lb = (rng.standard_normal((D_OUT, R), dtype=np.float32) * 0.02).astype(np.float32)
    out = kernel(x, w, b, la, lb)
    ref = x.reshape(-1, D_IN) @ (w + lb @ la).T + b
    err = np.abs(out.reshape(-1, D_OUT) - ref)
    denom = np.abs(ref).max()
    print("max abs err:", err.max(), "rel:", err.max() / denom)


# revision 15
# speedup vs baseline: 1.4478x; 1.1765x over previous
"""LoRA linear layer (out = x @ (W + B@A).T + bias) on 8 trn2 NeuronCores.

Strategy: data-parallel over tokens (B*S = 8192 -> 1024 tokens/core).
All layout work (transposes, dtype casts) happens on the host so the
device does nothing but matmuls:

  - x shard arrives pre-transposed: a fp8(e4m3) copy of the first K8
    contraction dims in DoubleRow pair layout, and a fp16 copy of the
    remaining dims. Both DMA straight into resident SBUF.
  - W arrives pre-transposed (k-major) and pre-cast: fp8 (DoubleRow
    [SW-interleave] pair layout, scaled by 64) for k < K8, fp16
    (scaled by 64) for k >= K8. Streamed per 128-row output block.
  - Per output block m and token chunk n: psum[o=128, t=512]
    accumulates G8 DoubleRow fp8 matmuls (256 k-dims each, 2x rate),
    KT16 fp16 matmuls (128 k-dims each), and one rank-16 LoRA matmul
    from B.T and U = (64*A) @ x.T.
  - Scalar engine evicts psum with the bias added and the 1/64 weight
    scale removed; plain contiguous DMA writes out.T [o, t] rows and
    the host transposes each shard back.

fp8 on only half the contraction dims keeps the max relative error
~1.8e-2 (measured on the reference data) while cutting PE time ~25%.
"""

import sys

sys.path.insert(0, "/opt/trn_rl_repo")

import numpy as np
import ml_dtypes

import concourse.bass as bass  # noqa: F401
import concourse.bacc as bacc
import concourse.tile as tile
from concourse import mybir, bass_utils
from contextlib import ExitStack

P = 128
N_CORES = 8

# Full problem shapes (hardcoded per contract).
B_FULL, S_FULL, D_IN, D_OUT, R = 4, 2048, 4096, 4096, 16
T_CORE = (B_FULL * S_FULL) // N_CORES  # 1024 tokens per core
MT = D_OUT // P  # 32 output row blocks
NCH = 512  # token chunk (one psum bank)
NT = T_CORE // NCH  # 2 chunks
K8 = 2048  # leading contraction dims done in fp8 DoubleRow
SCALE = 64.0  # fp8 weight scale (keeps 64*W in e4m3 normal range)
SWIL = True  # DoubleRowSwInterleave (contiguous ldweights) vs DoubleRow

FP8NP = ml_dtypes.float8_e4m3fn


def build_nc(T=T_CORE, k8=K8, swil=SWIL, fp16=None):
    """Per-core bass program; all cores run it on different token shards.

    fp16 kwarg is accepted for test.py compat: fp16=True/None keeps the
    default hybrid config, fp16-only can be forced with k8=0.
    """
    F32 = mybir.dt.float32
    F16 = mybir.dt.float16
    F8 = mybir.dt.float8e4
    G8 = k8 // 256
    KT16 = (D_IN - k8) // P
    DRMODE = (
        mybir.MatmulPerfMode.DoubleRowSwInterleave
        if swil
        else mybir.MatmulPerfMode.DoubleRow
    )
    IDENT = mybir.ActivationFunctionType.Identity
    HT = T // 2

    nc = bacc.Bacc("TRN2", target_bir_lowering=False, debug=False)
    if G8:
        # pair-interleaved moving layout: the two fp8 elements of each
        # DoubleRow pair sit adjacent in SBUF so the PE can stream both
        # per cycle
        x8_d = nc.dram_tensor("x8", [P, G8, T, 2], F8, kind="ExternalInput").ap()
        if swil:
            w8_d = nc.dram_tensor("w8", [MT, P, G8, 2 * P], F8, kind="ExternalInput").ap()
        else:
            w8_d = nc.dram_tensor("w8", [MT, P, G8, 2, P], F8, kind="ExternalInput").ap()
        at8_d = nc.dram_tensor("at8", [P, G8, 2, R], F8, kind="ExternalInput").ap()
    if KT16:
        x16_d = nc.dram_tensor("x16", [P, KT16, T], F16, kind="ExternalInput").ap()
        w16_d = nc.dram_tensor("w16", [MT, P, KT16, P], F16, kind="ExternalInput").ap()
        at16_d = nc.dram_tensor("at16", [P, KT16, R], F16, kind="ExternalInput").ap()
    bt_d = nc.dram_tensor("bt", [R, D_OUT], F16, kind="ExternalInput").ap()
    bias_d = nc.dram_tensor("bias_r", [P, MT], F32, kind="ExternalInput").ap()
    out_d = nc.dram_tensor("out", [D_OUT, T], F32, kind="ExternalOutput").ap()

    with tile.TileContext(nc) as tc, ExitStack() as ctx:
        const = ctx.enter_context(tc.tile_pool(name="const", bufs=1))
        if G8:
            xt8 = const.tile([P, G8, T, 2], F8)
            at8s = const.tile([P, G8, 2, R], F8)
        if KT16:
            xt16 = const.tile([P, KT16, T], F16)
            at16s = const.tile([P, KT16, R], F16)
        bt_sb = const.tile([R, D_OUT], F16)
        bias_sb = const.tile([P, MT], F32)
        u_sb = const.tile([R, T], F16)

        # ---- prologue DMAs: x halves spread over four queues ----
        if KT16:
            nc.sync.dma_start(xt16[:, :, 0:HT], x16_d[:, :, 0:HT])
            nc.gpsimd.dma_start(xt16[:, :, HT:T], x16_d[:, :, HT:T])
            nc.sync.dma_start(at16s[:], at16_d[:])
        if G8:
            nc.scalar.dma_start(xt8[:, :, 0:HT, :], x8_d[:, :, 0:HT, :])
            nc.scalar.dma_start(xt8[:, :, HT:T, :], x8_d[:, :, HT:T, :])
            nc.scalar.dma_start(at8s[:], at8_d[:])
        nc.sync.dma_start(bt_sb[:], bt_d[:])
        nc.sync.dma_start(bias_sb[:], bias_d[:])

        up_psum = ctx.enter_context(tc.tile_pool(name="upps", bufs=2, space="PSUM"))
        mm_psum = ctx.enter_context(tc.tile_pool(name="mmps", bufs=4, space="PSUM"))
        w8_pool = ctx.enter_context(tc.tile_pool(name="w8p", bufs=3))
        w16_pool = ctx.enter_context(tc.tile_pool(name="w16p", bufs=3))
        ob_pool = ctx.enter_context(tc.tile_pool(name="obp", bufs=4))

        # ---- U = (64*A) @ x^T, [R, T] ----
        for n in range(NT):
            nsl = slice(n * NCH, (n + 1) * NCH)
            ups = up_psum.tile([R, NCH], F32, tag="up", name="ups")
            first = True
            for g in range(G8):
                for s_ in range(2):
                    nc.tensor.matmul(
                        ups[:],
                        at8s[:, g, s_, :],
                        xt8[:, g, nsl, s_],
                        start=first,
                        stop=(not KT16) and g == G8 - 1 and s_ == 1,
                    )
                    first = False
            for j in range(KT16):
                nc.tensor.matmul(
                    ups[:],
                    at16s[:, j, :],
                    xt16[:, j, nsl],
                    start=first,
                    stop=j == KT16 - 1,
                )
                first = False
            nc.vector.tensor_copy(u_sb[:, nsl], ups[:])

        # ---- main loop over output row blocks, W prefetched one ahead ----
        def dma_w(m):
            t8 = t16 = None
            if G8:
                t8 = w8_pool.tile(
                    [P, G8, 2 * P] if swil else [P, G8, 2, P], F8, tag="w8", name="w8t"
                )
                nc.sync.dma_start(t8[:], w8_d[m])
            if KT16:
                t16 = w16_pool.tile([P, KT16, P], F16, tag="w16", name="w16t")
                nc.sync.dma_start(t16[:], w16_d[m])
            return t8, t16

        cur = dma_w(0)
        for m in range(MT):
            nxt = dma_w(m + 1) if m + 1 < MT else None
            w8t, w16t = cur
            msl = slice(m * P, (m + 1) * P)
            for n in range(NT):
                nsl = slice(n * NCH, (n + 1) * NCH)
                ps = mm_psum.tile([P, NCH], F32, tag="mm", name="mps")
                for g in range(G8):
                    w_ap = w8t[:, g, :] if swil else w8t[:, g, :, :]
                    nc.tensor.matmul(
                        ps[:],
                        w_ap,
                        xt8[:, g, nsl, :].rearrange("p t s -> p s t"),
                        start=g == 0,
                        stop=False,
                        perf_mode=DRMODE,
                    )
                for j in range(KT16):
                    nc.tensor.matmul(
                        ps[:],
                        w16t[:, j, :],
                        xt16[:, j, nsl],
                        start=(not G8) and j == 0,
                        stop=False,
                    )
                nc.tensor.matmul(
                    ps[:], bt_sb[:, msl], u_sb[:, nsl], start=False, stop=True
                )
                ob = ob_pool.tile([P, NCH], F32, tag="ob", name="ob")
                nc.scalar.activation(
                    ob[:], ps[:], IDENT, bias=bias_sb[:, m : m + 1], scale=1.0 / SCALE
                )
                (nc.scalar if n == 0 else nc.gpsimd).dma_start(out_d[msl, nsl], ob[:])
            cur = nxt

    nc.compile()
    return nc


def _fp8(a):
    return np.clip(a, -240.0, 240.0).astype(FP8NP)


def make_in_maps(x, weight, bias, lora_A, lora_B, k8=K8, swil=SWIL):
    G8 = k8 // 256
    KT16 = (D_IN - k8) // P
    T = T_CORE
    xf = np.ascontiguousarray(x.reshape(-1, D_IN), dtype=np.float32)

    # ---- shared (per-core identical) weight-side arrays ----
    shared = {}
    w = np.asarray(weight, dtype=np.float32)
    a = np.asarray(lora_A, dtype=np.float32)
    if G8:
        wq8 = _fp8(SCALE * w[:, :k8]).reshape(MT, P, G8, 2, P)
        if swil:
            # [m, p, g, c_rev, s] pairs, contiguous for SW-interleaved ldweights
            shared["w8"] = np.ascontiguousarray(
                wq8[:, ::-1].transpose(0, 4, 2, 1, 3).reshape(MT, P, G8, 2 * P)
            )
        else:
            shared["w8"] = np.ascontiguousarray(wq8.transpose(0, 4, 2, 3, 1))
        aq8 = _fp8(SCALE * a[:, :k8]).T.reshape(G8, 2, P, R)
        shared["at8"] = np.ascontiguousarray(aq8.transpose(2, 0, 1, 3))
    if KT16:
        w16 = (SCALE * w[:, k8:]).astype(np.float16).reshape(MT, P, KT16, P)
        shared["w16"] = np.ascontiguousarray(w16.transpose(0, 3, 2, 1))
        a16 = (SCALE * a[:, k8:]).astype(np.float16).T.reshape(KT16, P, R)
        shared["at16"] = np.ascontiguousarray(a16.transpose(1, 0, 2))
    shared["bt"] = np.ascontiguousarray(
        np.asarray(lora_B, dtype=np.float32).T.astype(np.float16)
    )
    shared["bias_r"] = np.ascontiguousarray(
        np.asarray(bias, dtype=np.float32).reshape(MT, P).T
    )

    # ---- per-core token shards ----
    maps = []
    for c in range(N_CORES):
        xs = np.ascontiguousarray(xf[c * T : (c + 1) * T].T)  # [D_IN, T]
        m = dict(shared)
        if G8:
            m["x8"] = np.ascontiguousarray(
                _fp8(xs[:k8]).reshape(G8, 2, P, T).transpose(2, 0, 3, 1)
            )
        if KT16:
            m["x16"] = np.ascontiguousarray(
                xs[k8:].astype(np.float16).reshape(KT16, P, T).transpose(1, 0, 2)
            )
        maps.append(m)
    return maps


_nc_cache = {}


def kernel(x, weight, bias, lora_A, lora_B):
    key = (x.shape, weight.shape)
    if key not in _nc_cache:
        _nc_cache[key] = build_nc()
    nc = _nc_cache[key]
    in_maps = make_in_maps(x, weight, bias, lora_A, lora_B)
    res = bass_utils.run_bass_kernel_spmd(
        nc, in_maps, core_ids=list(range(N_CORES))
    )
    out = np.concatenate(
        [res.results[c]["out"].T for c in range(N_CORES)], axis=0
    )
    return out.reshape(x.shape[:-1] + (weight.shape[0],))


if __name__ == "__main__":
    rng = np.random.default_rng(0)
    x = rng.standard_normal((B_FULL, S_FULL, D_IN), dtype=np.float32)
    w = (rng.standard_normal((D_OUT, D_IN), dtype=np.float32) * 0.02).astype(np.float32)
    b = (rng.standard_normal((D_OUT,), dtype=np.float32) * 0.02).astype(np.float32)
    la = (rng.standard_normal((R, D_IN), dtype=np.float32) * 0.02).astype(np.float32)
    lb = (rng.standard_normal((D_OUT, R), dtype=np.float32) * 0.02).astype(np.float32)
    out = kernel(x, w, b, la, lb)
    ref = x.reshape(-1, D_IN) @ (w + lb @ la).T + b
    err = np.abs(out.reshape(-1, D_OUT) - ref)
    denom = np.abs(ref).max()
    print("max abs err:", err.max(), "rel:", err.max() / denom)


# revision 20
# speedup vs baseline: 1.5028x; 1.0380x over previous
"""LoRA linear layer (out = x @ (W + B@A).T + bias) on 8 trn2 NeuronCores.

Strategy: data-parallel over tokens (B*S = 8192 -> 1024 tokens/core).
All layout work (transposes, dtype casts) happens on the host so the
device does nothing but matmuls:

  - x shard arrives pre-transposed: a fp8(e4m3) copy of the first K8
    contraction dims in DoubleRow pair layout, and a fp16 copy of the
    remaining dims. Both DMA straight into resident SBUF.
  - W arrives pre-transposed (k-major) and pre-cast: fp8 (DoubleRow
    [SW-interleave] pair layout, scaled by 64) for k < K8, fp16
    (scaled by 64) for k >= K8. Streamed per 128-row output block.
  - Per output block m and token chunk n: psum[o=128, t=512]
    accumulates G8 DoubleRow fp8 matmuls (256 k-dims each, 2x rate),
    KT16 fp16 matmuls (128 k-dims each), and one rank-16 LoRA matmul
    from B.T and U = (64*A) @ x.T.
  - Scalar engine evicts psum with the bias added and the 1/64 weight
    scale removed; plain contiguous DMA writes out.T [o, t] rows and
    the host transposes each shard back.

fp8 on only half the contraction dims keeps the max relative error
~1.8e-2 (measured on the reference data) while cutting PE time ~25%.
"""

import sys

sys.path.insert(0, "/opt/trn_rl_repo")

import numpy as np
import ml_dtypes

import concourse.bass as bass  # noqa: F401
import concourse.bacc as bacc
import concourse.tile as tile
from concourse import mybir, bass_utils
from contextlib import ExitStack

P = 128
N_CORES = 8

# Full problem shapes (hardcoded per contract).
B_FULL, S_FULL, D_IN, D_OUT, R = 4, 2048, 4096, 4096, 16
T_CORE = (B_FULL * S_FULL) // N_CORES  # 1024 tokens per core
MT = D_OUT // P  # 32 output row blocks
NCH = 512  # token chunk (one psum bank)
NT = T_CORE // NCH  # 2 chunks
K8 = 2304  # leading contraction dims done in fp8 DoubleRow
SCALE = 64.0  # fp8 weight scale (keeps 64*W in e4m3 normal range)
SWIL = True  # DoubleRowSwInterleave (contiguous ldweights) vs DoubleRow

FP8NP = ml_dtypes.float8_e4m3fn


def build_nc(T=T_CORE, k8=K8, swil=SWIL, fp16=None):
    """Per-core bass program; all cores run it on different token shards.

    fp16 kwarg is accepted for test.py compat: fp16=True/None keeps the
    default hybrid config, fp16-only can be forced with k8=0.
    """
    F32 = mybir.dt.float32
    F16 = mybir.dt.float16
    F8 = mybir.dt.float8e4
    G8 = k8 // 256
    KT16 = (D_IN - k8) // P
    DRMODE = (
        mybir.MatmulPerfMode.DoubleRowSwInterleave
        if swil
        else mybir.MatmulPerfMode.DoubleRow
    )
    IDENT = mybir.ActivationFunctionType.Identity
    HT = T // 2

    nc = bacc.Bacc("TRN2", target_bir_lowering=False, debug=False)
    if G8:
        # pair-interleaved moving layout: the two fp8 elements of each
        # DoubleRow pair sit adjacent in SBUF so the PE can stream both
        # per cycle
        x8_d = nc.dram_tensor("x8", [P, G8, T, 2], F8, kind="ExternalInput").ap()
        if swil:
            w8_d = nc.dram_tensor("w8", [MT, P, G8, 2 * P], F8, kind="ExternalInput").ap()
        else:
            w8_d = nc.dram_tensor("w8", [MT, P, G8, 2, P], F8, kind="ExternalInput").ap()
        at8_d = nc.dram_tensor("at8", [P, G8, 2, R], F8, kind="ExternalInput").ap()
    if KT16:
        x16_d = nc.dram_tensor("x16", [P, KT16, T], F16, kind="ExternalInput").ap()
        w16_d = nc.dram_tensor("w16", [MT, P, KT16, P], F16, kind="ExternalInput").ap()
        at16_d = nc.dram_tensor("at16", [P, KT16, R], F16, kind="ExternalInput").ap()
    bt_d = nc.dram_tensor("bt", [R, D_OUT], F16, kind="ExternalInput").ap()
    bias_d = nc.dram_tensor("bias_r", [P, MT], F32, kind="ExternalInput").ap()
    out_d = nc.dram_tensor("out", [D_OUT, T], F32, kind="ExternalOutput").ap()

    with tile.TileContext(nc) as tc, ExitStack() as ctx:
        const = ctx.enter_context(tc.tile_pool(name="const", bufs=1))
        if G8:
            xt8 = const.tile([P, G8, T, 2], F8)
            at8s = const.tile([P, G8, 2, R], F8)
        if KT16:
            xt16 = const.tile([P, KT16, T], F16)
            at16s = const.tile([P, KT16, R], F16)
        bt_sb = const.tile([R, D_OUT], F16)
        bias_sb = const.tile([P, MT], F32)
        u_sb = const.tile([R, T], F16)

        up_psum = ctx.enter_context(tc.tile_pool(name="upps", bufs=2, space="PSUM"))
        mm_psum = ctx.enter_context(tc.tile_pool(name="mmps", bufs=5, space="PSUM"))
        w8_pool = ctx.enter_context(tc.tile_pool(name="w8p", bufs=8))
        w16_pool = ctx.enter_context(tc.tile_pool(name="w16p", bufs=8))
        ob_pool = ctx.enter_context(tc.tile_pool(name="obp", bufs=6))

        w8_tiles, w16_tiles = {}, {}

        def dma_w8(m):
            t8 = w8_pool.tile(
                [P, G8, 2 * P] if swil else [P, G8, 2, P], F8, tag="w8", name="w8t"
            )
            nc.scalar.dma_start(t8[:], w8_d[m])
            w8_tiles[m] = t8

        def dma_w16(m):
            t16 = w16_pool.tile([P, KT16, P], F16, tag="w16", name="w16t")
            nc.sync.dma_start(t16[:], w16_d[m])
            w16_tiles[m] = t16

        # ---- prologue DMAs: explicit per-queue order so the pieces needed
        # by the first matmuls (x half 0, W block 0) land first ----
        QT = T // 4
        q = lambda i: slice(i * QT, (i + 1) * QT)
        nc.sync.dma_start(bias_sb[:], bias_d[:])
        nc.sync.dma_start(bt_sb[:], bt_d[:])
        if KT16:
            nc.sync.dma_start(at16s[:], at16_d[:])
        if G8:
            nc.sync.dma_start(at8s[:], at8_d[:])
            nc.scalar.dma_start(xt8[:, :, q(0), :], x8_d[:, :, q(0), :])
        if KT16:
            nc.sync.dma_start(xt16[:, :, q(0)], x16_d[:, :, q(0)])
            nc.gpsimd.dma_start(xt16[:, :, q(1)], x16_d[:, :, q(1)])
        if G8:
            dma_w8(0)
            nc.gpsimd.dma_start(xt8[:, :, q(1), :], x8_d[:, :, q(1), :])
        if KT16:
            dma_w16(0)
            nc.sync.dma_start(xt16[:, :, q(2)], x16_d[:, :, q(2)])
            nc.gpsimd.dma_start(xt16[:, :, q(3)], x16_d[:, :, q(3)])
        for m_ in (1, 2, 3):
            if G8:
                dma_w8(m_)
        if G8:
            nc.gpsimd.dma_start(xt8[:, :, q(2), :], x8_d[:, :, q(2), :])
            nc.gpsimd.dma_start(xt8[:, :, q(3), :], x8_d[:, :, q(3), :])
        for m_ in (1, 2, 3):
            if KT16:
                dma_w16(m_)

        # ---- U = (64*A) @ x^T, [R, T], one token chunk ----
        def emit_u(n):
            nsl = slice(n * NCH, (n + 1) * NCH)
            ups = up_psum.tile([R, NCH], F32, tag="up", name="ups")
            first = True
            for g in range(G8):
                for s_ in range(2):
                    nc.tensor.matmul(
                        ups[:],
                        at8s[:, g, s_, :],
                        xt8[:, g, nsl, s_],
                        start=first,
                        stop=(not KT16) and g == G8 - 1 and s_ == 1,
                    )
                    first = False
            for j in range(KT16):
                nc.tensor.matmul(
                    ups[:],
                    at16s[:, j, :],
                    xt16[:, j, nsl],
                    start=first,
                    stop=j == KT16 - 1,
                )
                first = False
            nc.vector.tensor_copy(u_sb[:, nsl], ups[:])

        # ---- one (m, n) output tile: 24 accumulating matmuls + eviction ----
        def emit_group(m, n):
            issued = w8_tiles if G8 else w16_tiles
            if n == 0 and m + 3 < MT and (m + 3) not in issued:
                if G8:
                    dma_w8(m + 3)
                if KT16:
                    dma_w16(m + 3)
            msl = slice(m * P, (m + 1) * P)
            nsl = slice(n * NCH, (n + 1) * NCH)
            ps = mm_psum.tile([P, NCH], F32, tag="mm", name="mps")
            for g in range(G8):
                w8t = w8_tiles[m]
                w_ap = w8t[:, g, :] if swil else w8t[:, g, :, :]
                nc.tensor.matmul(
                    ps[:],
                    w_ap,
                    xt8[:, g, nsl, :].rearrange("p t s -> p s t"),
                    start=g == 0,
                    stop=False,
                    perf_mode=DRMODE,
                )
            for j in range(KT16):
                nc.tensor.matmul(
                    ps[:],
                    w16_tiles[m][:, j, :],
                    xt16[:, j, nsl],
                    start=(not G8) and j == 0,
                    stop=False,
                )
            nc.tensor.matmul(
                ps[:], bt_sb[:, msl], u_sb[:, nsl], start=False, stop=True
            )
            ob = ob_pool.tile([P, NCH], F32, tag="ob", name="ob")
            nc.scalar.activation(
                ob[:], ps[:], IDENT, bias=bias_sb[:, m : m + 1], scale=1.0 / SCALE
            )
            if m == MT - 1:
                e0, e1 = (nc.sync, nc.gpsimd) if n == 0 else (nc.sync, nc.scalar)
                HN = NCH // 2
                n0 = n * NCH
                e0.dma_start(out_d[msl, n0 : n0 + HN], ob[:, 0:HN])
                e1.dma_start(out_d[msl, n0 + HN : n0 + NCH], ob[:, HN:NCH])
            else:
                eng = (nc.gpsimd, nc.sync, nc.scalar)[(2 * m + n) % 3]
                eng.dma_start(out_d[msl, nsl], ob[:])

        # warm order: n=0 tiles of the first four blocks run before any n=1
        # tile so compute starts as soon as the first half of x has landed
        WARM = min(4, MT)
        emit_u(0)
        for m in range(WARM):
            emit_group(m, 0)
        emit_u(1)
        for m in range(WARM):
            emit_group(m, 1)
        for m in range(WARM, MT):
            emit_group(m, 0)
            emit_group(m, 1)

    nc.compile()
    return nc


def _fp8(a):
    return np.clip(a, -240.0, 240.0).astype(FP8NP)


def make_in_maps(x, weight, bias, lora_A, lora_B, k8=K8, swil=SWIL):
    G8 = k8 // 256
    KT16 = (D_IN - k8) // P
    T = T_CORE
    xf = np.ascontiguousarray(x.reshape(-1, D_IN), dtype=np.float32)

    # ---- shared (per-core identical) weight-side arrays ----
    shared = {}
    w = np.asarray(weight, dtype=np.float32)
    a = np.asarray(lora_A, dtype=np.float32)
    if G8:
        wq8 = _fp8(SCALE * w[:, :k8]).reshape(MT, P, G8, 2, P)
        if swil:
            # [m, p, g, c_rev, s] pairs, contiguous for SW-interleaved ldweights
            shared["w8"] = np.ascontiguousarray(
                wq8[:, ::-1].transpose(0, 4, 2, 1, 3).reshape(MT, P, G8, 2 * P)
            )
        else:
            shared["w8"] = np.ascontiguousarray(wq8.transpose(0, 4, 2, 3, 1))
        aq8 = _fp8(SCALE * a[:, :k8]).T.reshape(G8, 2, P, R)
        shared["at8"] = np.ascontiguousarray(aq8.transpose(2, 0, 1, 3))
    if KT16:
        w16 = (SCALE * w[:, k8:]).astype(np.float16).reshape(MT, P, KT16, P)
        shared["w16"] = np.ascontiguousarray(w16.transpose(0, 3, 2, 1))
        a16 = (SCALE * a[:, k8:]).astype(np.float16).T.reshape(KT16, P, R)
        shared["at16"] = np.ascontiguousarray(a16.transpose(1, 0, 2))
    shared["bt"] = np.ascontiguousarray(
        np.asarray(lora_B, dtype=np.float32).T.astype(np.float16)
    )
    shared["bias_r"] = np.ascontiguousarray(
        np.asarray(bias, dtype=np.float32).reshape(MT, P).T
    )

    # ---- per-core token shards ----
    maps = []
    for c in range(N_CORES):
        xs = np.ascontiguousarray(xf[c * T : (c + 1) * T].T)  # [D_IN, T]
        m = dict(shared)
        if G8:
            m["x8"] = np.ascontiguousarray(
                _fp8(xs[:k8]).reshape(G8, 2, P, T).transpose(2, 0, 3, 1)
            )
        if KT16:
            m["x16"] = np.ascontiguousarray(
                xs[k8:].astype(np.float16).reshape(KT16, P, T).transpose(1, 0, 2)
            )
        maps.append(m)
    return maps


_nc_cache = {}


def kernel(x, weight, bias, lora_A, lora_B):
    key = (x.shape, weight.shape)
    if key not in _nc_cache:
        _nc_cache[key] = build_nc()
    nc = _nc_cache[key]
    in_maps = make_in_maps(x, weight, bias, lora_A, lora_B)
    res = bass_utils.run_bass_kernel_spmd(
        nc, in_maps, core_ids=list(range(N_CORES))
    )
    out = np.concatenate(
        [res.results[c]["out"].T for c in range(N_CORES)], axis=0
    )
    return out.reshape(x.shape[:-1] + (weight.shape[0],))


if __name__ == "__main__":
    rng = np.random.default_rng(0)
    x = rng.standard_normal((B_FULL, S_FULL, D_IN), dtype=np.float32)
    w = (rng.standard_normal((D_OUT, D_IN), dtype=np.float32) * 0.02).astype(np.float32)
    b = (rng.standard_normal((D_OUT,), dtype=np.float32) * 0.02).astype(np.float32)
    la = (rng.standard_normal((R, D_IN), dtype=np.float32) * 0.02).astype(np.float32)
    lb = (rng.standard_normal((D_OUT, R), dtype=np.float32) * 0.02).astype(np.float32)
    out = kernel(x, w, b, la, lb)
    ref = x.reshape(-1, D_IN) @ (w + lb @ la).T + b
    err = np.abs(out.reshape(-1, D_OUT) - ref)
    denom = np.abs(ref).max()
    print("max abs err:", err.max(), "rel:", err.max() / denom)


# revision 23
# speedup vs baseline: 1.5140x; 1.0075x over previous
"""LoRA linear layer (out = x @ (W + B@A).T + bias) on 8 trn2 NeuronCores.

Strategy: data-parallel over tokens (B*S = 8192 -> 1024 tokens/core).
All layout work (transposes, dtype casts) happens on the host so the
device does nothing but matmuls:

  - x shard arrives pre-transposed: a fp8(e4m3) copy of the first K8
    contraction dims in DoubleRow pair layout, and a fp16 copy of the
    remaining dims. Both DMA straight into resident SBUF.
  - W arrives pre-transposed (k-major) and pre-cast: fp8 (DoubleRow
    [SW-interleave] pair layout, scaled by 64) for k < K8, fp16
    (scaled by 64) for k >= K8. Streamed per 128-row output block.
  - Per output block m and token chunk n: psum[o=128, t=512]
    accumulates G8 DoubleRow fp8 matmuls (256 k-dims each, 2x rate),
    KT16 fp16 matmuls (128 k-dims each), and one rank-16 LoRA matmul
    from B.T and U = (64*A) @ x.T.
  - Scalar engine evicts psum with the bias added and the 1/64 weight
    scale removed; plain contiguous DMA writes out.T [o, t] rows and
    the host transposes each shard back.

fp8 on only half the contraction dims keeps the max relative error
~1.8e-2 (measured on the reference data) while cutting PE time ~25%.
"""

import sys

sys.path.insert(0, "/opt/trn_rl_repo")

import numpy as np
import ml_dtypes

import concourse.bass as bass  # noqa: F401
import concourse.bacc as bacc
import concourse.tile as tile
from concourse import mybir, bass_utils
from contextlib import ExitStack

P = 128
N_CORES = 8

# Full problem shapes (hardcoded per contract).
B_FULL, S_FULL, D_IN, D_OUT, R = 4, 2048, 4096, 4096, 16
T_CORE = (B_FULL * S_FULL) // N_CORES  # 1024 tokens per core
MT = D_OUT // P  # 32 output row blocks
NCH = 512  # token chunk (one psum bank)
NT = T_CORE // NCH  # 2 chunks
K8 = 2304  # leading contraction dims done in fp8 DoubleRow
SCALE = 64.0  # fp8 weight scale (keeps 64*W in e4m3 normal range)
SWIL = True  # DoubleRowSwInterleave (contiguous ldweights) vs DoubleRow

FP8NP = ml_dtypes.float8_e4m3fn


def build_nc(T=T_CORE, k8=K8, swil=SWIL, fp16=None):
    """Per-core bass program; all cores run it on different token shards.

    fp16 kwarg is accepted for test.py compat: fp16=True/None keeps the
    default hybrid config, fp16-only can be forced with k8=0.
    """
    F32 = mybir.dt.float32
    F16 = mybir.dt.float16
    F8 = mybir.dt.float8e4
    G8 = k8 // 256
    KT16 = (D_IN - k8) // P
    DRMODE = (
        mybir.MatmulPerfMode.DoubleRowSwInterleave
        if swil
        else mybir.MatmulPerfMode.DoubleRow
    )
    IDENT = mybir.ActivationFunctionType.Identity
    HT = T // 2

    nc = bacc.Bacc("TRN2", target_bir_lowering=False, debug=False)
    if G8:
        # pair-interleaved moving layout: the two fp8 elements of each
        # DoubleRow pair sit adjacent in SBUF so the PE can stream both
        # per cycle
        x8_d = nc.dram_tensor("x8", [P, G8, T, 2], F8, kind="ExternalInput").ap()
        if swil:
            w8_d = nc.dram_tensor("w8", [MT, P, G8, 2 * P], F8, kind="ExternalInput").ap()
        else:
            w8_d = nc.dram_tensor("w8", [MT, P, G8, 2, P], F8, kind="ExternalInput").ap()
        at8_d = nc.dram_tensor("at8", [P, G8, 2, R], F8, kind="ExternalInput").ap()
    if KT16:
        x16_d = nc.dram_tensor("x16", [P, KT16, T], F16, kind="ExternalInput").ap()
        w16_d = nc.dram_tensor("w16", [MT, P, KT16, P], F16, kind="ExternalInput").ap()
        at16_d = nc.dram_tensor("at16", [P, KT16, R], F16, kind="ExternalInput").ap()
    bt_d = nc.dram_tensor("bt", [R, D_OUT], F16, kind="ExternalInput").ap()
    bias_d = nc.dram_tensor("bias_r", [P, MT], F32, kind="ExternalInput").ap()
    out_d = nc.dram_tensor("out", [D_OUT, T], F32, kind="ExternalOutput").ap()

    with tile.TileContext(nc) as tc, ExitStack() as ctx:
        const = ctx.enter_context(tc.tile_pool(name="const", bufs=1))
        if G8:
            xt8 = const.tile([P, G8, T, 2], F8)
            at8s = const.tile([P, G8, 2, R], F8)
        if KT16:
            xt16 = const.tile([P, KT16, T], F16)
            at16s = const.tile([P, KT16, R], F16)
        bt_sb = const.tile([R, D_OUT], F16)
        bias_sb = const.tile([P, MT], F32)
        u_sb = const.tile([R, T], F16)

        up_psum = ctx.enter_context(tc.tile_pool(name="upps", bufs=2, space="PSUM"))
        mm_psum = ctx.enter_context(tc.tile_pool(name="mmps", bufs=6, space="PSUM"))
        w8_pool = ctx.enter_context(tc.tile_pool(name="w8p", bufs=8))
        w16_pool = ctx.enter_context(tc.tile_pool(name="w16p", bufs=8))
        ob_pool = ctx.enter_context(tc.tile_pool(name="obp", bufs=6))

        w8_tiles, w16_tiles = {}, {}

        def dma_w8(m):
            t8 = w8_pool.tile(
                [P, G8, 2 * P] if swil else [P, G8, 2, P], F8, tag="w8", name="w8t"
            )
            nc.scalar.dma_start(t8[:], w8_d[m])
            w8_tiles[m] = t8

        def dma_w16(m):
            t16 = w16_pool.tile([P, KT16, P], F16, tag="w16", name="w16t")
            nc.sync.dma_start(t16[:], w16_d[m])
            w16_tiles[m] = t16

        # ---- prologue DMAs: x split by k-range across the three queues so
        # every transfer keeps 2KB+ contiguous per-partition lines; the
        # first output block's matmuls chase the arriving k-tiles ----
        nc.sync.dma_start(bias_sb[:], bias_d[:])
        nc.sync.dma_start(bt_sb[:], bt_d[:])
        if KT16:
            nc.sync.dma_start(at16s[:], at16_d[:])
        if G8:
            nc.sync.dma_start(at8s[:], at8_d[:])
            dma_w8(0)
        if KT16:
            dma_w16(0)
        if G8:
            for g in range(G8 - 1):
                nc.scalar.dma_start(xt8[:, g], x8_d[:, g])
            nc.gpsimd.dma_start(xt8[:, G8 - 1], x8_d[:, G8 - 1])
        if KT16:
            JH = KT16 // 2
            nc.sync.dma_start(xt16[:, 0:JH, :], x16_d[:, 0:JH, :])
            nc.gpsimd.dma_start(xt16[:, JH:KT16, :], x16_d[:, JH:KT16, :])
        for m_ in (1, 2, 3):
            if m_ < MT:
                if G8:
                    dma_w8(m_)
                if KT16:
                    dma_w16(m_)

        # ---- U = (64*A) @ x^T, [R, T], one token chunk ----
        def emit_u(n):
            nsl = slice(n * NCH, (n + 1) * NCH)
            ups = up_psum.tile([R, NCH], F32, tag="up", name="ups")
            first = True
            for g in range(G8):
                for s_ in range(2):
                    nc.tensor.matmul(
                        ups[:],
                        at8s[:, g, s_, :],
                        xt8[:, g, nsl, s_],
                        start=first,
                        stop=(not KT16) and g == G8 - 1 and s_ == 1,
                    )
                    first = False
            for j in range(KT16):
                nc.tensor.matmul(
                    ups[:],
                    at16s[:, j, :],
                    xt16[:, j, nsl],
                    start=first,
                    stop=j == KT16 - 1,
                )
                first = False
            nc.vector.tensor_copy(u_sb[:, nsl], ups[:])

        # ---- one (m, n) output tile: 24 accumulating matmuls + eviction ----
        def emit_main(m, n):
            issued = w8_tiles if G8 else w16_tiles
            if n == 0 and m + 3 < MT and (m + 3) not in issued:
                if G8:
                    dma_w8(m + 3)
                if KT16:
                    dma_w16(m + 3)
            nsl = slice(n * NCH, (n + 1) * NCH)
            ps = mm_psum.tile([P, NCH], F32, tag="mm", name="mps")
            for g in range(G8):
                w8t = w8_tiles[m]
                w_ap = w8t[:, g, :] if swil else w8t[:, g, :, :]
                nc.tensor.matmul(
                    ps[:],
                    w_ap,
                    xt8[:, g, nsl, :].rearrange("p t s -> p s t"),
                    start=g == 0,
                    stop=False,
                    perf_mode=DRMODE,
                )
            for j in range(KT16):
                nc.tensor.matmul(
                    ps[:],
                    w16_tiles[m][:, j, :],
                    xt16[:, j, nsl],
                    start=(not G8) and j == 0,
                    stop=False,
                )
            return ps

        def emit_fin(m, n, ps):
            msl = slice(m * P, (m + 1) * P)
            nsl = slice(n * NCH, (n + 1) * NCH)
            nc.tensor.matmul(
                ps[:], bt_sb[:, msl], u_sb[:, nsl], start=False, stop=True
            )
            ob = ob_pool.tile([P, NCH], F32, tag="ob", name="ob")
            nc.scalar.activation(
                ob[:], ps[:], IDENT, bias=bias_sb[:, m : m + 1], scale=1.0 / SCALE
            )
            if m == MT - 1:
                e0, e1 = (nc.sync, nc.gpsimd) if n == 0 else (nc.sync, nc.scalar)
                HN = NCH // 2
                n0 = n * NCH
                e0.dma_start(out_d[msl, n0 : n0 + HN], ob[:, 0:HN])
                e1.dma_start(out_d[msl, n0 + HN : n0 + NCH], ob[:, HN:NCH])
            else:
                eng = nc.gpsimd if (2 * m + n) % 2 == 0 else nc.sync
                eng.dma_start(out_d[msl, nsl], ob[:])

        # first block's matmuls chase the prologue DMAs; U slots in behind
        # them (it needs all of x) and before the first LoRA matmul
        ps00 = emit_main(0, 0)
        emit_u(0)
        emit_fin(0, 0, ps00)
        ps01 = emit_main(0, 1)
        emit_u(1)
        emit_fin(0, 1, ps01)
        for m in range(1, MT):
            for n in range(NT):
                emit_fin(m, n, emit_main(m, n))

    nc.compile()
    return nc


def _fp8(a):
    return np.clip(a, -240.0, 240.0).astype(FP8NP)


def make_in_maps(x, weight, bias, lora_A, lora_B, k8=K8, swil=SWIL):
    G8 = k8 // 256
    KT16 = (D_IN - k8) // P
    T = T_CORE
    xf = np.ascontiguousarray(x.reshape(-1, D_IN), dtype=np.float32)

    # ---- shared (per-core identical) weight-side arrays ----
    shared = {}
    w = np.asarray(weight, dtype=np.float32)
    a = np.asarray(lora_A, dtype=np.float32)
    if G8:
        wq8 = _fp8(SCALE * w[:, :k8]).reshape(MT, P, G8, 2, P)
        if swil:
            # [m, p, g, c_rev, s] pairs, contiguous for SW-interleaved ldweights
            shared["w8"] = np.ascontiguousarray(
                wq8[:, ::-1].transpose(0, 4, 2, 1, 3).reshape(MT, P, G8, 2 * P)
            )
        else:
            shared["w8"] = np.ascontiguousarray(wq8.transpose(0, 4, 2, 3, 1))
        aq8 = _fp8(SCALE * a[:, :k8]).T.reshape(G8, 2, P, R)
        shared["at8"] = np.ascontiguousarray(aq8.transpose(2, 0, 1, 3))
    if KT16:
        w16 = (SCALE * w[:, k8:]).astype(np.float16).reshape(MT, P, KT16, P)
        shared["w16"] = np.ascontiguousarray(w16.transpose(0, 3, 2, 1))
        a16 = (SCALE * a[:, k8:]).astype(np.float16).T.reshape(KT16, P, R)
        shared["at16"] = np.ascontiguousarray(a16.transpose(1, 0, 2))
    shared["bt"] = np.ascontiguousarray(
        np.asarray(lora_B, dtype=np.float32).T.astype(np.float16)
    )
    shared["bias_r"] = np.ascontiguousarray(
        np.asarray(bias, dtype=np.float32).reshape(MT, P).T
    )

    # ---- per-core token shards ----
    maps = []
    for c in range(N_CORES):
        xs = np.ascontiguousarray(xf[c * T : (c + 1) * T].T)  # [D_IN, T]
        m = dict(shared)
        if G8:
            m["x8"] = np.ascontiguousarray(
                _fp8(xs[:k8]).reshape(G8, 2, P, T).transpose(2, 0, 3, 1)
            )
        if KT16:
            m["x16"] = np.ascontiguousarray(
                xs[k8:].astype(np.float16).reshape(KT16, P, T).transpose(1, 0, 2)
            )
        maps.append(m)
    return maps


_nc_cache = {}


def kernel(x, weight, bias, lora_A, lora_B):
    key = (x.shape, weight.shape)
    if key not in _nc_cache:
        _nc_cache[key] = build_nc()
    nc = _nc_cache[key]
    in_maps = make_in_maps(x, weight, bias, lora_A, lora_B)
    res = bass_utils.run_bass_kernel_spmd(
        nc, in_maps, core_ids=list(range(N_CORES))
    )
    out = np.concatenate(
        [res.results[c]["out"].T for c in range(N_CORES)], axis=0
    )
    return out.reshape(x.shape[:-1] + (weight.shape[0],))


if __name__ == "__main__":
    rng = np.random.default_rng(0)
    x = rng.standard_normal((B_FULL, S_FULL, D_IN), dtype=np.float32)
    w = (rng.standard_normal((D_OUT, D_IN), dtype=np.float32) * 0.02).astype(np.float32)
    b = (rng.standard_normal((D_OUT,), dtype=np.float32) * 0.02).astype(np.float32)
    la = (rng.standard_normal((R, D_IN), dtype=np.float32) * 0.02).astype(np.float32)
    lb = (rng.standard_normal((D_OUT, R), dtype=np.float32) * 0.02).astype(np.float32)
    out = kernel(x, w, b, la, lb)
    ref = x.reshape(-1, D_IN) @ (w + lb @ la).T + b
    err = np.abs(out.reshape(-1, D_OUT) - ref)
    denom = np.abs(ref).max()
    print("max abs err:", err.max(), "rel:", err.max() / denom)


# revision 26
# speedup vs baseline: 1.5481x; 1.0225x over previous
"""LoRA linear layer (out = x @ (W + B@A).T + bias) on 8 trn2 NeuronCores.

Strategy: data-parallel over tokens (B*S = 8192 -> 1024 tokens/core).
All layout work (transposes, dtype casts) happens on the host so the
device does nothing but matmuls:

  - x shard arrives pre-transposed: a fp8(e4m3) copy of the first K8
    contraction dims in DoubleRow pair layout, and a fp16 copy of the
    remaining dims. Both DMA straight into resident SBUF.
  - W arrives pre-transposed (k-major) and pre-cast: fp8 (DoubleRow
    [SW-interleave] pair layout, scaled by 64) for k < K8, fp16
    (scaled by 64) for k >= K8. Streamed per 128-row output block.
  - Per output block m and token chunk n: psum[o=128, t=512]
    accumulates G8 DoubleRow fp8 matmuls (256 k-dims each, 2x rate),
    KT16 fp16 matmuls (128 k-dims each), and one rank-16 LoRA matmul
    from B.T and U = (64*A) @ x.T.
  - Scalar engine evicts psum with the bias added and the 1/64 weight
    scale removed; plain contiguous DMA writes out.T [o, t] rows and
    the host transposes each shard back.

fp8 on only half the contraction dims keeps the max relative error
~1.8e-2 (measured on the reference data) while cutting PE time ~25%.
"""

import sys

sys.path.insert(0, "/opt/trn_rl_repo")

import numpy as np
import ml_dtypes

import concourse.bass as bass  # noqa: F401
import concourse.bacc as bacc
import concourse.tile as tile
from concourse import mybir, bass_utils
from contextlib import ExitStack

P = 128
N_CORES = 8

# Full problem shapes (hardcoded per contract).
B_FULL, S_FULL, D_IN, D_OUT, R = 4, 2048, 4096, 4096, 16
T_CORE = (B_FULL * S_FULL) // N_CORES  # 1024 tokens per core
MT = D_OUT // P  # 32 output row blocks
NCH = 512  # token chunk (one psum bank)
NT = T_CORE // NCH  # 2 chunks
K8 = 2304  # leading contraction dims done in fp8 DoubleRow
SCALE = 64.0  # fp8 weight scale (keeps 64*W in e4m3 normal range)
SWIL = True  # DoubleRowSwInterleave (contiguous ldweights) vs DoubleRow

FP8NP = ml_dtypes.float8_e4m3fn


def build_nc(T=T_CORE, k8=K8, swil=SWIL, fp16=None):
    """Per-core bass program; all cores run it on different token shards.

    fp16 kwarg is accepted for test.py compat: fp16=True/None keeps the
    default hybrid config, fp16-only can be forced with k8=0.
    """
    F32 = mybir.dt.float32
    F16 = mybir.dt.float16
    F8 = mybir.dt.float8e4
    G8 = k8 // 256
    KT16 = (D_IN - k8) // P
    DRMODE = (
        mybir.MatmulPerfMode.DoubleRowSwInterleave
        if swil
        else mybir.MatmulPerfMode.DoubleRow
    )
    IDENT = mybir.ActivationFunctionType.Identity
    HT = T // 2

    nc = bacc.Bacc("TRN2", target_bir_lowering=False, debug=False)
    if G8:
        # pair-interleaved moving layout: the two fp8 elements of each
        # DoubleRow pair sit adjacent in SBUF so the PE can stream both
        # per cycle
        x8_d = nc.dram_tensor("x8", [P, G8, T, 2], F8, kind="ExternalInput").ap()
        if swil:
            w8_d = nc.dram_tensor("w8", [MT, P, G8, 2 * P], F8, kind="ExternalInput").ap()
        else:
            w8_d = nc.dram_tensor("w8", [MT, P, G8, 2, P], F8, kind="ExternalInput").ap()
        at8_d = nc.dram_tensor("at8", [P, G8, 2, R], F8, kind="ExternalInput").ap()
    if KT16:
        x16_d = nc.dram_tensor("x16", [P, KT16, T], F16, kind="ExternalInput").ap()
        w16_d = nc.dram_tensor("w16", [MT, P, KT16, P], F16, kind="ExternalInput").ap()
        at16_d = nc.dram_tensor("at16", [P, KT16, R], F16, kind="ExternalInput").ap()
    bt_d = nc.dram_tensor("bt", [R, D_OUT], F16, kind="ExternalInput").ap()
    bias_d = nc.dram_tensor("bias_r", [P, MT], F32, kind="ExternalInput").ap()
    out_d = nc.dram_tensor("out", [D_OUT, T], F32, kind="ExternalOutput").ap()

    with tile.TileContext(nc) as tc, ExitStack() as ctx:
        const = ctx.enter_context(tc.tile_pool(name="const", bufs=1))
        if G8:
            xt8 = const.tile([P, G8, T, 2], F8)
            at8s = const.tile([P, G8, 2, R], F8)
        if KT16:
            xt16 = const.tile([P, KT16, T], F16)
            at16s = const.tile([P, KT16, R], F16)
        bt_sb = const.tile([R, D_OUT], F16)
        bias_sb = const.tile([P, MT], F32)
        u_sb = const.tile([R, T], F16)

        up_psum = ctx.enter_context(tc.tile_pool(name="upps", bufs=2, space="PSUM"))
        mm_psum = ctx.enter_context(tc.tile_pool(name="mmps", bufs=6, space="PSUM"))
        w8_pool = ctx.enter_context(tc.tile_pool(name="w8p", bufs=8))
        w16_pool = ctx.enter_context(tc.tile_pool(name="w16p", bufs=8))
        ob_pool = ctx.enter_context(tc.tile_pool(name="obp", bufs=6))

        w8_tiles, w16_tiles = {}, {}

        def dma_w8(m):
            t8 = w8_pool.tile(
                [P, G8, 2 * P] if swil else [P, G8, 2, P], F8, tag="w8", name="w8t"
            )
            nc.scalar.dma_start(t8[:], w8_d[m])
            w8_tiles[m] = t8

        def dma_w16(m):
            t16 = w16_pool.tile([P, KT16, P], F16, tag="w16", name="w16t")
            nc.sync.dma_start(t16[:], w16_d[m])
            w16_tiles[m] = t16

        # ---- prologue DMAs: x split by k-range across the three queues so
        # every transfer keeps 2KB+ contiguous per-partition lines; the
        # first output block's matmuls chase the arriving k-tiles ----
        nc.sync.dma_start(bias_sb[:], bias_d[:])
        nc.sync.dma_start(bt_sb[:], bt_d[:])
        if KT16:
            nc.sync.dma_start(at16s[:], at16_d[:])
        if G8:
            nc.sync.dma_start(at8s[:], at8_d[:])
            dma_w8(0)
        if KT16:
            dma_w16(0)
        if G8:
            for g in range(G8 - 1):
                nc.scalar.dma_start(xt8[:, g], x8_d[:, g])
            nc.gpsimd.dma_start(xt8[:, G8 - 1], x8_d[:, G8 - 1])
        if KT16:
            JH = KT16 // 2
            nc.sync.dma_start(xt16[:, 0:JH, :], x16_d[:, 0:JH, :])
            nc.gpsimd.dma_start(xt16[:, JH:KT16, :], x16_d[:, JH:KT16, :])
        if 1 < MT:
            if G8:
                dma_w8(1)
            if KT16:
                dma_w16(1)

        # ---- U = (64*A) @ x^T, [R, T], one token chunk ----
        def emit_u(n):
            nsl = slice(n * NCH, (n + 1) * NCH)
            ups = up_psum.tile([R, NCH], F32, tag="up", name="ups")
            first = True
            for g in range(G8):
                for s_ in range(2):
                    nc.tensor.matmul(
                        ups[:],
                        at8s[:, g, s_, :],
                        xt8[:, g, nsl, s_],
                        start=first,
                        stop=(not KT16) and g == G8 - 1 and s_ == 1,
                    )
                    first = False
            for j in range(KT16):
                nc.tensor.matmul(
                    ups[:],
                    at16s[:, j, :],
                    xt16[:, j, nsl],
                    start=first,
                    stop=j == KT16 - 1,
                )
                first = False
            nc.vector.tensor_copy(u_sb[:, nsl], ups[:])

        # ---- one (m, n) output tile: 24 accumulating matmuls + eviction ----
        def emit_main(m, n):
            issued = w8_tiles if G8 else w16_tiles
            if n == 0:
                for mw in (m + 2, m + 3):
                    if mw < MT and mw not in issued:
                        if G8:
                            dma_w8(mw)
                        if KT16:
                            dma_w16(mw)
                        break
            nsl = slice(n * NCH, (n + 1) * NCH)
            ps = mm_psum.tile([P, NCH], F32, tag="mm", name="mps")
            for g in range(G8):
                w8t = w8_tiles[m]
                w_ap = w8t[:, g, :] if swil else w8t[:, g, :, :]
                nc.tensor.matmul(
                    ps[:],
                    w_ap,
                    xt8[:, g, nsl, :].rearrange("p t s -> p s t"),
                    start=g == 0,
                    stop=False,
                    perf_mode=DRMODE,
                )
            for j in range(KT16):
                nc.tensor.matmul(
                    ps[:],
                    w16_tiles[m][:, j, :],
                    xt16[:, j, nsl],
                    start=(not G8) and j == 0,
                    stop=False,
                )
            return ps

        def emit_fin(m, n, ps):
            msl = slice(m * P, (m + 1) * P)
            nsl = slice(n * NCH, (n + 1) * NCH)
            nc.tensor.matmul(
                ps[:], bt_sb[:, msl], u_sb[:, nsl], start=False, stop=True
            )
            ob = ob_pool.tile([P, NCH], F32, tag="ob", name="ob")
            nc.scalar.activation(
                ob[:], ps[:], IDENT, bias=bias_sb[:, m : m + 1], scale=1.0 / SCALE
            )
            if m == MT - 1:
                e0, e1 = (nc.sync, nc.gpsimd) if n == 0 else (nc.sync, nc.scalar)
                HN = NCH // 2
                n0 = n * NCH
                e0.dma_start(out_d[msl, n0 : n0 + HN], ob[:, 0:HN])
                e1.dma_start(out_d[msl, n0 + HN : n0 + NCH], ob[:, HN:NCH])
            else:
                eng = nc.gpsimd if (2 * m + n) % 2 == 0 else nc.sync
                eng.dma_start(out_d[msl, nsl], ob[:])

        # first blocks' matmuls chase the prologue DMAs; U (which needs all
        # of x) runs after them and before the first LoRA matmuls
        ps0 = [emit_main(0, n) for n in range(NT)]
        ps1 = [emit_main(1, n) for n in range(NT)] if MT > 1 else []
        emit_u(0)
        emit_u(1)
        for n in range(NT):
            emit_fin(0, n, ps0[n])
        for n in range(NT):
            if ps1:
                emit_fin(1, n, ps1[n])
        for m in range(2, MT):
            for n in range(NT):
                emit_fin(m, n, emit_main(m, n))

    nc.compile()
    return nc


def _fp8(a):
    return np.clip(a, -240.0, 240.0).astype(FP8NP)


def make_in_maps(x, weight, bias, lora_A, lora_B, k8=K8, swil=SWIL):
    G8 = k8 // 256
    KT16 = (D_IN - k8) // P
    T = T_CORE
    xf = np.ascontiguousarray(x.reshape(-1, D_IN), dtype=np.float32)

    # ---- shared (per-core identical) weight-side arrays ----
    shared = {}
    w = np.asarray(weight, dtype=np.float32)
    a = np.asarray(lora_A, dtype=np.float32)
    if G8:
        wq8 = _fp8(SCALE * w[:, :k8]).reshape(MT, P, G8, 2, P)
        if swil:
            # [m, p, g, c_rev, s] pairs, contiguous for SW-interleaved ldweights
            shared["w8"] = np.ascontiguousarray(
                wq8[:, ::-1].transpose(0, 4, 2, 1, 3).reshape(MT, P, G8, 2 * P)
            )
        else:
            shared["w8"] = np.ascontiguousarray(wq8.transpose(0, 4, 2, 3, 1))
        aq8 = _fp8(SCALE * a[:, :k8]).T.reshape(G8, 2, P, R)
        shared["at8"] = np.ascontiguousarray(aq8.transpose(2, 0, 1, 3))
    if KT16:
        w16 = (SCALE * w[:, k8:]).astype(np.float16).reshape(MT, P, KT16, P)
        shared["w16"] = np.ascontiguousarray(w16.transpose(0, 3, 2, 1))
        a16 = (SCALE * a[:, k8:]).astype(np.float16).T.reshape(KT16, P, R)
        shared["at16"] = np.ascontiguousarray(a16.transpose(1, 0, 2))
    shared["bt"] = np.ascontiguousarray(
        np.asarray(lora_B, dtype=np.float32).T.astype(np.float16)
    )
    shared["bias_r"] = np.ascontiguousarray(
        np.asarray(bias, dtype=np.float32).reshape(MT, P).T
    )

    # ---- per-core token shards ----
    maps = []
    for c in range(N_CORES):
        xs = np.ascontiguousarray(xf[c * T : (c + 1) * T].T)  # [D_IN, T]
        m = dict(shared)
        if G8:
            m["x8"] = np.ascontiguousarray(
                _fp8(xs[:k8]).reshape(G8, 2, P, T).transpose(2, 0, 3, 1)
            )
        if KT16:
            m["x16"] = np.ascontiguousarray(
                xs[k8:].astype(np.float16).reshape(KT16, P, T).transpose(1, 0, 2)
            )
        maps.append(m)
    return maps


_nc_cache = {}


def kernel(x, weight, bias, lora_A, lora_B):
    key = (x.shape, weight.shape)
    if key not in _nc_cache:
        _nc_cache[key] = build_nc()
    nc = _nc_cache[key]
    in_maps = make_in_maps(x, weight, bias, lora_A, lora_B)
    res = bass_utils.run_bass_kernel_spmd(
        nc, in_maps, core_ids=list(range(N_CORES))
    )
    out = np.concatenate(
        [res.results[c]["out"].T for c in range(N_CORES)], axis=0
    )
    return out.reshape(x.shape[:-1] + (weight.shape[0],))


if __name__ == "__main__":
    rng = np.random.default_rng(0)
    x = rng.standard_normal((B_FULL, S_FULL, D_IN), dtype=np.float32)
    w = (rng.standard_normal((D_OUT, D_IN), dtype=np.float32) * 0.02).astype(np.float32)
    b = (rng.standard_normal((D_OUT,), dtype=np.float32) * 0.02).astype(np.float32)
    la = (rng.standard_normal((R, D_IN), dtype=np.float32) * 0.02).astype(np.float32)
    lb = (rng.standard_normal((D_OUT, R), dtype=np.float32) * 0.02).astype(np.float32)
    out = kernel(x, w, b, la, lb)
    ref = x.reshape(-1, D_IN) @ (w + lb @ la).T + b
    err = np.abs(out.reshape(-1, D_OUT) - ref)
    denom = np.abs(ref).max()
    print("max abs err:", err.max(), "rel:", err.max() / denom)
